# revision 14
# baseline (speedup 1.0000x reference)
"""Mixtral-style MoE (8 experts, top-2) on 8 Trainium2 NeuronCores.

Strategy: expert parallelism. The router (gate matmul + top-2 + renorm) is
computed on host in float64 (it is tiny: T x H x E). Tokens are gathered
per-expert on host, padded to a common capacity C, and each core runs the
full SwiGLU FFN for one expert in bf16 (fp32 PSUM accumulation), applying
the per-token combine weight on device. Host scatters-adds the two expert
contributions per token back into the full [T, H] output.

Device layouts (per core, expert e):
  xt  [H, C]            bf16  tokens for this expert, transposed (h-major)
  w1t [I/128, 128, H]   bf16  w1[e] tiled: [ib, p, hc*128+i] = w1[e][ib*128+i, hc*128+p]
  w3t same as w1t
  w2t [I/128, 128, H]   bf16  w2[e].T row-chunks: [ic, p, h] = w2[e][h, ic*128+p]
  scl [128, C/128]      f32   combine weight, token t = n*128+p at [p, n]
  y   [C, H]            f32   scaled expert output
"""
import sys

sys.path.insert(0, "/opt/trn_rl_repo")
from contextlib import ExitStack

import ml_dtypes
import numpy as np

T = 16384
H = 1024
I = 3584
E = 8
NCORES = 8
TCH = 512  # token chunk = matmul moving dim

_prog_cache = {}


def _build_program(C):
    import concourse.bacc as bacc
    import concourse.bass as bass
    import concourse.mybir as mybir
    import concourse.tile as tile

    f32 = mybir.dt.float32
    bf16 = mybir.dt.bfloat16
    NIB = I // 128  # 28 inter blocks
    NHC = H // 128  # 8 hidden chunks
    NHH = H // 512  # out col chunks
    assert C % 128 == 0
    chunks = [TCH] * (C // TCH)
    if C % TCH:
        chunks.append(C % TCH)

    nc = bacc.Bacc("TRN2", target_bir_lowering=False, debug=False, num_devices=NCORES)
    xt = nc.dram_tensor("xt", [H, C], bf16, kind="ExternalInput").ap()
    w1t = nc.dram_tensor("w1t", [NIB, 128, H], bf16, kind="ExternalInput").ap()
    w3t = nc.dram_tensor("w3t", [NIB, 128, H], bf16, kind="ExternalInput").ap()
    w2t = nc.dram_tensor("w2t", [NIB, 128, H], bf16, kind="ExternalInput").ap()
    scl = nc.dram_tensor("scl", [128, C // 128], f32, kind="ExternalInput").ap()
    y = nc.dram_tensor("y", [C, H], f32, kind="ExternalOutput").ap()

    with tile.TileContext(nc) as tc, ExitStack() as ctx:
        w2pool = ctx.enter_context(tc.tile_pool(name="w2pool", bufs=1))
        sclpool = ctx.enter_context(tc.tile_pool(name="sclpool", bufs=1))
        xpool = ctx.enter_context(tc.tile_pool(name="xpool", bufs=2))
        wpool = ctx.enter_context(tc.tile_pool(name="wpool", bufs=4))
        tpool = ctx.enter_context(tc.tile_pool(name="tpool", bufs=3))
        apool = ctx.enter_context(tc.tile_pool(name="apool", bufs=2))
        ypool = ctx.enter_context(tc.tile_pool(name="ypool", bufs=3))
        pa = ctx.enter_context(tc.tile_pool(name="pa", bufs=2, space="PSUM"))
        pb = ctx.enter_context(tc.tile_pool(name="pb", bufs=2, space="PSUM"))

        sclsb = sclpool.tile([128, C // 128], f32, name="sclsb")
        nc.sync.dma_start(sclsb, scl)

        w2sb = []
        t0 = 0
        for ch, tch in enumerate(chunks):
            w0 = None
            if ch == 0:
                # head: first i-block's weights before the x tiles so the
                # first matmul's inputs (w1[0], x[0]) arrive earliest
                w1sb0 = wpool.tile([128, H], bf16, tag="w1", name="w1sb0")
                nc.sync.dma_start(w1sb0, w1t[0])
                w3sb0 = wpool.tile([128, H], bf16, tag="w3", name="w3sb0")
                nc.sync.dma_start(w3sb0, w3t[0])
                w0 = (w1sb0, w3sb0)
            xsb = []
            for hc in range(NHC):
                xc = xpool.tile([128, TCH], bf16, tag=f"x{hc}", name=f"xsb{hc}")[
                    :, :tch
                ]
                nc.sync.dma_start(xc, xt[hc * 128 : (hc + 1) * 128, t0 : t0 + tch])
                xsb.append(xc)
            a_tiles = []
            for ib in range(NIB):
                if ib == 0 and w0 is not None:
                    w1sb, w3sb = w0
                else:
                    w1sb = wpool.tile([128, H], bf16, tag="w1", name="w1sb")
                    nc.sync.dma_start(w1sb, w1t[ib])
                    w3sb = wpool.tile([128, H], bf16, tag="w3", name="w3sb")
                    nc.sync.dma_start(w3sb, w3t[ib])
                psh1 = pa.tile([128, TCH], f32, tag="h1", name="psh1")[:, :tch]
                psh3 = pa.tile([128, TCH], f32, tag="h3", name="psh3")[:, :tch]
                for hc in range(NHC):
                    nc.tensor.matmul(
                        psh1,
                        w1sb[:, hc * 128 : (hc + 1) * 128],
                        xsb[hc],
                        start=(hc == 0),
                        stop=(hc == NHC - 1),
                    )
                for hc in range(NHC):
                    nc.tensor.matmul(
                        psh3,
                        w3sb[:, hc * 128 : (hc + 1) * 128],
                        xsb[hc],
                        start=(hc == 0),
                        stop=(hc == NHC - 1),
                    )
                tmp = tpool.tile([128, TCH], f32, tag="silu", name="tmp")[:, :tch]
                nc.scalar.activation(tmp, psh1, mybir.ActivationFunctionType.Silu)
                ab = apool.tile([128, TCH], bf16, tag=f"a{ib}", name=f"ab{ib}")[
                    :, :tch
                ]
                nc.vector.tensor_mul(ab, tmp, psh3)
                a_tiles.append(ab)
            if ch == 0:
                # w2 loads emitted after chunk-0 phase A so they don't delay
                # the first matmuls (only needed for phase B, ~150us in)
                for ic in range(NIB):
                    w2c = w2pool.tile([128, H], bf16, tag=f"w2_{ic}", name=f"w2sb{ic}")
                    nc.sync.dma_start(w2c, w2t[ic])
                    w2sb.append(w2c)
            for tt in range(tch // 128):
                tglob = t0 // 128 + tt
                for hh in range(NHH):
                    psy = pb.tile([128, 512], f32, tag="y", name="psy")
                    for ic in range(NIB):
                        nc.tensor.matmul(
                            psy,
                            a_tiles[ic][:, tt * 128 : (tt + 1) * 128],
                            w2sb[ic][:, hh * 512 : (hh + 1) * 512],
                            start=(ic == 0),
                            stop=(ic == NIB - 1),
                        )
                    ysb = ypool.tile([128, 512], f32, tag="ysb", name="ysb")
                    nc.scalar.mul(ysb, psy, sclsb[:, tglob : tglob + 1])
                    nc.sync.dma_start(
                        y[tglob * 128 : (tglob + 1) * 128, hh * 512 : (hh + 1) * 512],
                        ysb,
                    )
            t0 += tch
    nc.compile()
    return nc


def _route(hidden_states, gate_w):
    """Host router. Returns per-expert token indices and combine weights."""
    logits = hidden_states.astype(np.float64) @ gate_w.astype(np.float64).T  # [T, E]
    ar = np.arange(logits.shape[0])
    i0 = np.argmax(logits, axis=1)
    l0 = logits[ar, i0]
    masked = logits.copy()
    masked[ar, i0] = -np.inf
    i1 = np.argmax(masked, axis=1)
    l1 = masked[ar, i1]
    # renormalized top-2 softmax weights (full-softmax denominator cancels)
    wt0 = 1.0 / (1.0 + np.exp(l1 - l0))
    wt1 = 1.0 - wt0
    idx = []
    wts = []
    for e in range(E):
        sel = np.flatnonzero((i0 == e) | (i1 == e))
        idx.append(sel)
        wts.append(np.where(i0[sel] == e, wt0[sel], wt1[sel]).astype(np.float32))
    return idx, wts


def kernel(hidden_states, gate_w, w1, w3, w2):
    from concourse import bass_utils

    hs = np.ascontiguousarray(np.asarray(hidden_states, dtype=np.float32))
    gw = np.asarray(gate_w, dtype=np.float32)
    w1 = np.asarray(w1, dtype=np.float32)
    w3 = np.asarray(w3, dtype=np.float32)
    w2 = np.asarray(w2, dtype=np.float32)

    idx, wts = _route(hs, gw)
    NIB = I // 128
    NHC = H // 128
    maxn = max(len(s) for s in idx)
    C = max(128, ((maxn + 127) // 128) * 128)

    if C not in _prog_cache:
        _prog_cache[C] = _build_program(C)
    nc = _prog_cache[C]

    in_maps = []
    for e in range(E):
        n = len(idx[e])
        xt = np.zeros((H, C), dtype=ml_dtypes.bfloat16)
        xt[:, :n] = hs[idx[e]].T.astype(ml_dtypes.bfloat16)
        scl = np.zeros((C,), dtype=np.float32)
        scl[:n] = wts[e]
        scl = np.ascontiguousarray(scl.reshape(C // 128, 128).T)
        w1t = np.ascontiguousarray(
            w1[e].reshape(NIB, 128, NHC, 128).transpose(0, 3, 2, 1).reshape(NIB, 128, H)
        ).astype(ml_dtypes.bfloat16)
        w3t = np.ascontiguousarray(
            w3[e].reshape(NIB, 128, NHC, 128).transpose(0, 3, 2, 1).reshape(NIB, 128, H)
        ).astype(ml_dtypes.bfloat16)
        w2t = np.ascontiguousarray(w2[e].T.reshape(NIB, 128, H)).astype(
            ml_dtypes.bfloat16
        )
        in_maps.append({"xt": xt, "w1t": w1t, "w3t": w3t, "w2t": w2t, "scl": scl})

    res = bass_utils.run_bass_kernel_spmd(nc, in_maps, core_ids=list(range(NCORES)))

    out = np.zeros((T, H), dtype=np.float32)
    for e in range(E):
        n = len(idx[e])
        out[idx[e]] += res.results[e]["y"][:n]
    return out
